# revision 35
# baseline (speedup 1.0000x reference)
"""Causal multi-head attention (B=2, S=2048, D=768, H=12) on 8 Trainium2 cores.

Sharding: core c -> batch b = c//4, head-group g = c%4 (heads 3g..3g+2).
Each core computes its 3 heads end-to-end in bf16 (fp32 PSUM accumulation)
and produces a partial output-projection y_partial[b] = out_g @ Wo_g^T
(+ bo on g==0 cores).  The host sums the 4 partials per batch (the
"all-reduce") while unsharding.

Device layout notes (per core):
  qkT groups (dim-on-partition, token-on-free), each [128, 2048] bf16:
    G0 = [q_h0 (p0-63) ; q_h1 (p64-127)]
    G1 = [k_h0 ; k_h1]
    G2 = [q_h2 ; k_h2]
  q_h2 is DMA-rehomed to partitions 64-127 and k_h2 to partitions 0-63 so
  head-2 score matmuls can alternate between PE row-groups (load balance
  against heads 0/1 which are pinned to row groups 0 and 64).
  Scores are computed transposed S_T[k, q] so the softmax denominator
  falls out of the AV matmul via a ones column appended to v.  The
  denominator row is reciprocal'd in fp32 on one lane and DMA-broadcast
  across partitions for the normalization multiply.

v2 changes vs baseline:
  - input DMAs split across sync/scalar HWDGE + gpsimd SWDGE queues,
    critical tensors (wqk g0, x chunk 0) first -> first matmul ~5us earlier
  - dummy warm-up matmuls during the DMA wait so the PE HAM clock-gate
    lifts (2.4GHz) before real work starts
  - normalization broadcast moved off the PE (ones-matmul) onto the DMA
    engines (stride-0 partition broadcast of the fp32 reciprocal row)
  - out-projections of chunk j interleave into chunk j+2's attention steps
    so the output DMA drains during compute instead of a cold tail
"""

import os
import sys

import numpy as np

for _p in ("/opt/trn_rl_repo",):
    if _p not in sys.path and os.path.isdir(_p):
        sys.path.insert(0, _p)

import ml_dtypes  # noqa: E402

import concourse.bass as bass  # noqa: E402
import concourse.mybir as mybir  # noqa: E402
import concourse.tile as tile  # noqa: E402
from concourse import bacc  # noqa: E402
from concourse.bass_utils import run_bass_kernel_spmd  # noqa: E402
from concourse.tile_rust import add_dep_helper  # noqa: E402

BF16 = mybir.dt.bfloat16
F32 = mybir.dt.float32
NPBF = ml_dtypes.bfloat16

B, S, D = 2, 2048, 768
H, HD = 12, 64
NCORE = 8
HPC = 3  # heads per core
FT = D // 128  # 6 contraction tiles for projections
ST = S // 128  # 16 token tiles
QC = S // 512  # 4 q-chunks of 512
SCALE = float(1.0 / np.sqrt(HD))

_CACHE: dict = {}

MASK_GP = os.environ.get("MASK_GP", "0") == "1"
NORM_PE = os.environ.get("NORM_PE", "1") == "1"  # ones-matmul bcast (default)
WARM = int(os.environ.get("WARM", "28"))  # warm-up dummy matmuls
BC_BIG = os.environ.get("BC_BIG", "0") == "1"  # norm-bcast tiles in ps_big
TAILWARM = int(os.environ.get("TAILWARM", "14"))  # tail keep-warm matmuls


def _emit(nc: bacc.Bacc, tc: tile.TileContext, dr: dict, y_dr) -> None:
    from contextlib import ExitStack

    Exp = mybir.ActivationFunctionType.Exp

    with ExitStack() as ex:
        pool = lambda name, bufs, space="SBUF": ex.enter_context(  # noqa: E731
            tc.tile_pool(name=name, bufs=bufs, space=space)
        )

        consts = pool("consts", 1)

        # ---- persistent SBUF tensors -------------------------------------
        xT = consts.tile([128, QC, FT, 512], BF16)  # x[b]^T, chunk-major
        wqk = consts.tile([128, 3, FT, 128], BF16)  # qk proj lhsT tiles (g-major)
        wv = consts.tile([128, FT, 192], BF16)  # v projection rhs tiles
        woAB = consts.tile([128, D], BF16)  # out-proj rhs, heads 0+1 packed
        woC = consts.tile([65, D], BF16)  # out-proj rhs, head 2 (+bias row)
        bqk = consts.tile([128, 3], F32)
        bv = consts.tile([128, 192], F32)
        mask = consts.tile([128, 128], BF16)  # tri mask m[p,c]=1 if p<=c
        ones = consts.tile([65, 64], mybir.dt.float16)  # bcast matmul lhsT (row 64)
        dummy = consts.tile([128, 640], BF16)  # warm-up matmul operand

        qkT = consts.tile([128, 3, S], BF16)  # projected q/k groups
        qCmv = consts.tile([128, S], BF16)  # q_h2 rehomed to partitions 64-127
        kCmv = consts.tile([128, S], BF16)  # k_h2 rehomed to partitions 0-63
        vsb = consts.tile([128, ST, HPC, 65], BF16)  # v (+ones col) per ktile
        outAB = consts.tile([128, S], BF16)  # normalized out_T heads 0 (+1 moved)
        outC = consts.tile([65, S], BF16)  # normalized out_T head 2 (+ones row)
        outB = consts.tile([64, S], BF16)  # normalized out_T head 1 (pre-move)

        # ---- input streaming ----------------------------------------------
        # Three DMA issue queues run in parallel (sync HWDGE, scalar HWDGE,
        # gpsimd SWDGE).  Critical-path tensors go first: the first
        # projection matmul needs wqk[g0] + all of x chunk 0.
        xview = dr["xT"].ap().rearrange("p (c f s) -> p c f s", c=QC, f=FT)
        wqkview = dr["wqk"].ap().rearrange("p (g f m) -> p g f m", g=3, f=FT)
        # sync HWDGE (lowest latency ~1.5us): wqk g0 then the two contiguous
        # x chunk-0 halves -- exactly what the first projection group needs
        nc.sync.dma_start(out=wqk[:, 0], in_=wqkview[:, 0])
        nc.sync.dma_start(out=xT[:, 0, 0:3, :], in_=xview[:, 0, 0:3, :])
        nc.sync.dma_start(out=xT[:, 0, 3:6, :], in_=xview[:, 0, 3:6, :])
        # gpsimd SWDGE (~2.5us latency) carries the rest of the weights
        nc.gpsimd.dma_start(out=wqk[:, 1], in_=wqkview[:, 1])
        nc.gpsimd.dma_start(out=wqk[:, 2], in_=wqkview[:, 2])
        nc.gpsimd.dma_start(
            out=wv[:], in_=dr["wv"].ap().rearrange("p (f m) -> p f m", f=FT)
        )
        nc.gpsimd.dma_start(out=bv[:], in_=dr["bv"].ap())
        nc.gpsimd.dma_start(out=woAB[:], in_=dr["woAB"].ap())
        nc.gpsimd.dma_start(out=woC[:], in_=dr["woC"].ap())
        # scalar HWDGE: small early-needed tensors only (its ring is slow)
        nc.scalar.dma_start(out=bqk[:], in_=dr["bqk"].ap())
        nc.scalar.dma_start(out=mask[:], in_=dr["mask"].ap())
        for c in range(1, QC):
            nc.sync.dma_start(out=xT[:, c, :, :], in_=xview[:, c, :, :])
        nc.vector.memset(dummy[:], 0.0)
        nc.vector.memset(vsb[:, :, :, 64:65], 1.0)
        nc.vector.memset(outC[64:65, :], 1.0)  # bias row for out-proj
        if NORM_PE:
            nc.vector.memset(ones[64:65, :], 1.0)

        # ---- PSUM pools (8 banks total, statically allocated) -------------
        # ps_big: 2 slots x 2 banks  -> qk-proj, v-proj, scores(A,B), out-proj
        # ps_av : 3 slots x 1 bank   -> AV accumulators
        # ps_sm : 1 slot  x 1 bank   -> scores(C)
        ps_big = pool("ps_big", 2, "PSUM")
        ps_av = pool("ps_av", 3, "PSUM")
        ps_sm = pool("ps_sm", 1, "PSUM")

        # ---- PE warm-up ---------------------------------------------------
        # The HAM clock gate starts every kernel at 1.2GHz and only lifts to
        # 2.4GHz after ~3.4us of sustained PE activity.  Issue dummy matmuls
        # (zero data, result never read) while the input DMAs are in flight
        # so the real matmuls start at full clock.
        if WARM:
            ps_warm = ps_big.tile([128, 1024], F32, tag="big", name="warm")
            for k in range(WARM):
                # last few dummies are 512-wide: they bridge input-DMA
                # arrival jitter without delaying real work when data is
                # early (each is only ~0.2-0.4us of in-order queue)
                rhs = dummy[:, 0:128] if k < WARM - 8 else dummy[:, 128:640]
                nc.tensor.matmul(
                    ps_warm[:, 0 : rhs.shape[-1]], lhsT=dummy[:, 0:128], rhs=rhs,
                    start=True, stop=True,
                )

        # ---- projection pieces (emitted interleaved with attention) --------
        def emit_proj_qk(q4, g):
            qs = slice(512 * q4, 512 * (q4 + 1))
            ps = ps_big.tile([128, 1024], F32, tag="big", name=f"qkp_{g}_{q4}")
            for f in range(FT):
                nc.tensor.matmul(
                    ps[:, 0:512],
                    lhsT=wqk[:, g, f, :],
                    rhs=xT[:, q4, f, :],
                    start=(f == 0),
                    stop=(f == FT - 1),
                )
            nc.vector.tensor_scalar_add(qkT[:, g, qs], ps[:, 0:512], bqk[:, g : g + 1])
            if g == 2:
                # rehome head-2 q/k so C-scores can run on either row-group
                nc.sync.dma_start(out=qCmv[64:128, qs], in_=qkT[0:64, 2, qs])
                nc.sync.dma_start(out=kCmv[0:64, qs], in_=qkT[64:128, 2, qs])

        def emit_proj_v(st):
            ps = ps_big.tile([128, 1024], F32, tag="big", name=f"vp_{st}")
            c, sub = st // 4, st % 4
            for f in range(FT):
                nc.tensor.matmul(
                    ps[:, 0:192],
                    lhsT=xT[:, c, f, 128 * sub : 128 * (sub + 1)],
                    rhs=wv[:, f, :],
                    start=(f == 0),
                    stop=(f == FT - 1),
                )
            nc.vector.tensor_add(
                vsb[:, st, :, 0:64],
                ps[:, 0:192].rearrange("p (h d) -> p h d", h=3),
                bv[:].rearrange("p (h d) -> p h d", h=3),
            )

        def proj_pieces(c):
            out = [lambda g=g: emit_proj_qk(c, g) for g in range(3)]
            out += [lambda st=st: emit_proj_v(st) for st in range(4 * c, 4 * c + 4)]
            return out

        # ---- attention -----------------------------------------------------
        exp_sb = pool("exp_sb", 7)
        den_sb = pool("den_sb", 2)
        rec_sb = pool("rec_sb", 3)

        def emit_scores(j, i):
            off = 128 * (i - 4 * j) if i >= 4 * j else 0
            qs = slice(512 * j + off, 512 * (j + 1))
            ks = slice(128 * i, 128 * (i + 1))
            sAB_raw = ps_big.tile([128, 1024], F32, tag="big", name=f"sAB_{j}_{i}")
            sAB = sAB_raw[:].rearrange("p (h q) -> p h q", h=2)
            sC = ps_sm.tile([128, 512], F32, tag="sm", name=f"sC_{j}_{i}")
            mmA = nc.tensor.matmul(
                sAB[:, 0, off:], lhsT=qkT[0:64, 1, ks], rhs=qkT[0:64, 0, qs]
            )
            if i == 0:
                gate.clear()
            if i < 2:
                gate.append(mmA)
            nc.tensor.matmul(
                sAB[:, 1, off:], lhsT=qkT[64:128, 1, ks], rhs=qkT[64:128, 0, qs]
            )
            if i % 2 == 0:
                nc.tensor.matmul(sC[:, off:], lhsT=kCmv[0:64, ks], rhs=qkT[0:64, 2, qs])
            else:
                nc.tensor.matmul(
                    sC[:, off:], lhsT=qkT[64:128, 2, ks], rhs=qCmv[64:128, qs]
                )
            es = exp_sb.tile([128, HPC, 512], BF16, tag="es", name=f"es_{j}_{i}")
            if off == 0:
                # contiguous fast path: flat 1-D APs for the A|B pair
                nc.scalar.activation(
                    es[:].rearrange("p h q -> p (h q)")[:, 0:1024],
                    sAB_raw[:, 0:1024],
                    Exp,
                    scale=SCALE,
                )
            else:
                nc.scalar.activation(es[:, 0:2, off:], sAB[:, :, off:], Exp, scale=SCALE)
            nc.scalar.activation(es[:, 2, off:], sC[:, off:], Exp, scale=SCALE)
            return es

        def emit_av(j, i, nk, es, av):
            off = 128 * (i - 4 * j) if i >= 4 * j else 0
            if i >= 4 * j:  # diagonal block: zero the k>q half
                dm = slice(off, off + 128)
                meng = nc.gpsimd if MASK_GP else nc.vector
                meng.tensor_mul(
                    es[:, :, dm],
                    es[:, :, dm],
                    mask[:, None, :].broadcast_to([128, HPC, 128]),
                )
            for h in range(HPC):
                mm = nc.tensor.matmul(
                    av[h][:, off:],
                    lhsT=vsb[:, i, h, :],
                    rhs=es[:, h, off:],
                    start=(i == 0),
                    stop=(i == nk - 1),
                )
                # at the last chunk boundary, let its first scores beat the
                # prior chunk's AV backlog onto the PE stream (ordering only)
                if i >= nk - 3 and gate:
                    for gmm in gate:
                        add_dep_helper(
                            mm.ins, gmm.ins, sync=False,
                            reason="boundary: scores before AV backlog",
                        )

        def emit_norm(j, av):
            qs_full = slice(512 * j, 512 * (j + 1))
            # normalization: out = outU * (1/denom) ; denom = av row 64.
            # head 1 first: its result needs a rehome DMA before the packed
            # out-proj, so start that while heads 0/2 normalize
            if not NORM_PE:
                # fp32 reciprocal of the denominator row, then DMA partition-
                # broadcast (stride-0 src) to rows 0-63 -- no PE involvement
                for h in (1, 0, 2):
                    rec = rec_sb.tile([65, 512], F32, tag="rec", name=f"rc_{j}_{h}")
                    nc.vector.reciprocal_approx_fast(
                        rec[64:65, :], av[h][64:65, :]
                    )
                    nc.sync.dma_start(out=rec[0:1, :], in_=rec[64:65, :])
                    nc.gpsimd.partition_broadcast(rec[0:64, :], rec[0:1, :])
                    dst = (outAB[0:64, qs_full], outB[:, qs_full], outC[0:64, qs_full])[h]
                    nc.vector.tensor_mul(dst, av[h][0:64, :], rec[0:64, :])
                    if h == 1:
                        # move head-1 slice onto partitions 64-127 for out-proj
                        nc.sync.dma_start(
                            out=outAB[64:128, qs_full], in_=outB[:, qs_full]
                        )
                return
            dens = []
            for h in range(HPC):
                den = den_sb.tile(
                    [65, 512], mybir.dt.float16, tag="den", name=f"dn_{j}_{h}"
                )
                nc.vector.tensor_copy(den[64:65, 0:512], av[h][64:65, :])
                dens.append(den)
            # h0 first so its av PSUM slot (the one the next chunk's first
            # AV allocation waits on) frees earliest
            for h in (0, 1, 2):
                # bc tiles in ps_big (BC_BIG=1): sharing the single ps_sm
                # bank serializes the next chunk's C-scores behind the whole
                # norm chain at every chunk boundary
                if BC_BIG:
                    bc_t = ps_big.tile([128, 1024], F32, tag="big", name=f"b_{j}_{h}")
                    bc = bc_t[:, 0:512]
                else:
                    bc = ps_sm.tile([128, 512], F32, tag="sm", name=f"b_{j}_{h}")
                nc.tensor.matmul(
                    bc[0:64, :], lhsT=ones[64:65, :], rhs=dens[h][64:65, 0:512]
                )
                rec = rec_sb.tile([64, 512], F32, tag="rec", name=f"rc_{j}_{h}")
                nc.vector.reciprocal_approx_fast(rec[:], bc[0:64, :])
                dst = (outAB[0:64, qs_full], outB[:, qs_full], outC[0:64, qs_full])[h]
                nc.vector.tensor_mul(dst, av[h][0:64, :], rec[:])
                if h == 1:
                    nc.sync.dma_start(out=outAB[64:128, qs_full], in_=outB[:, qs_full])

        y_sb = pool("y_sb", 3)
        y_view = y_dr.ap().rearrange("(st p) e -> st p e", p=128)

        def emit_oproj(st):
            ss = slice(128 * st, 128 * (st + 1))
            ysb = y_sb.tile([128, D], F32, tag="ysb", name=f"ysb_{st}")
            ps = ps_big.tile([128, 1024], F32, tag="big", name=f"yp_{st}")
            for n0, nw in ((0, 512), (512, 256)):
                nc.tensor.matmul(
                    ps[:, n0 : n0 + nw],
                    lhsT=outAB[:, ss],
                    rhs=woAB[:, n0 : n0 + nw],
                    start=True,
                    stop=False,
                )
                nc.tensor.matmul(
                    ps[:, n0 : n0 + nw],
                    lhsT=outC[:, ss],
                    rhs=woC[:, n0 : n0 + nw],
                    start=False,
                    stop=True,
                )
            if st % 2 == 0:
                nc.vector.tensor_copy(ysb[:], ps[:, 0:D])
                nc.sync.dma_start(out=y_view[st], in_=ysb[:])
            else:
                nc.scalar.copy(ysb[:], ps[:, 0:D])
                nc.gpsimd.dma_start(out=y_view[st], in_=ysb[:])

        # flat software pipeline over all (j, i) steps: scores/exp run LAG
        # steps ahead of AV, crossing chunk boundaries so neither PE nor ACT
        # drains at chunk turns.  Norms are delayed NDELAY further steps so
        # they never gate the scores stream; each chunk's out-projection
        # stiles trickle into later chunks' step streams (one stile every
        # OP_EVERY steps) so the output DMA drains during compute.
        LAG = int(os.environ.get("LAG", "3"))
        NDELAY = int(os.environ.get("NDELAY", "0"))
        OP_EVERY = int(os.environ.get("OP_EVERY", "2"))
        # steps between a chunk's norm emission and its first out-proj stile:
        # the norm chain (copy/bcast/recip/mul + head-1 rehome DMA) must have
        # completed by then or the oproj matmuls head-of-line-block the
        # in-order PE queue at exactly the chunk boundary
        OP_DELAY = int(os.environ.get("OP_DELAY", "5"))
        # the first AV steps of each chunk get extra lag: their PSUM
        # accumulator allocation waits on the previous chunk's norm chain,
        # and with plain LAG that wait lands at the head of the in-order PE
        # queue exactly at the chunk boundary (observed HAM re-throttle)
        AV_EXTRA = int(os.environ.get("AV_EXTRA", "2"))
        steps = [(j, i) for j in range(QC) for i in range(4 * (j + 1))]
        av_sched: dict = {}
        for _idx, (_j, _i) in enumerate(steps):
            extra = max(0, AV_EXTRA - _i) if _j > 0 else 0
            av_sched.setdefault(_idx + LAG + extra, []).append(_idx)
        av_of: dict = {}
        es_of: dict = {}
        gate: list = []
        work_q: list = []  # deferred (kind, arg, delay-steps) emissions

        def do_av(idx):
            pj, pi = steps[idx]
            nkp = 4 * (pj + 1)
            if pi == 0:
                av_of[pj] = [
                    ps_av.tile([65, 512], F32, tag="av", name=f"av_{pj}_{h}")
                    for h in range(HPC)
                ]
            emit_av(pj, pi, nkp, es_of.pop((pj, pi)), av_of[pj])
            if pi == nkp - 1:
                work_q.append(("norm", pj, NDELAY))

        def drain_work_q():
            rest = []
            for kind, arg, delay in work_q:
                if delay > 0:
                    rest.append((kind, arg, delay - 1))
                    continue
                if kind == "norm":
                    emit_norm(arg, av_of.pop(arg))
                    for k in range(4):
                        rest.append(
                            ("oproj", 4 * arg + k, OP_DELAY + OP_EVERY * k)
                        )
                elif kind == "oproj":
                    emit_oproj(arg)
            work_q[:] = rest

        # proj chunk c+1's pieces are spread across attention chunk c's steps
        # (attention chunk j only needs projection chunks <= j).
        for piece in proj_pieces(0):
            piece()
        pend_proj: list = list(proj_pieces(1))
        for idx, (j, i) in enumerate(steps):
            nk = 4 * (j + 1)
            if pend_proj:
                want = max(1, -(-len(pend_proj) // max(1, nk - i)))
                for _ in range(want):
                    if pend_proj:
                        pend_proj.pop(0)()
            if i == nk - 1 and j + 2 < QC:
                pend_proj = list(proj_pieces(j + 2))
            es_of[(j, i)] = emit_scores(j, i)
            for t in av_sched.pop(idx, []):
                do_av(t)
            drain_work_q()
        for idx2 in sorted(av_sched):
            for t in av_sched[idx2]:
                do_av(t)
            drain_work_q()
        av_sched.clear()
        # keep the PE busy (and the HAM clock-gate open) through the last
        # chunk's norm chain so the final out-proj stiles run at full clock;
        # these sit between the norm broadcasts and the out-proj matmuls so
        # they fill exactly the DVE-bound normalization window
        if TAILWARM:
            ps_tw = ps_big.tile([128, 1024], F32, tag="big", name="tailwarm")
            for _ in range(TAILWARM):
                nc.tensor.matmul(
                    ps_tw[:, 0:512], lhsT=dummy[:, 0:128], rhs=dummy[:, 128:640],
                    start=True, stop=True,
                )
        while work_q:
            drain_work_q()


def _build():
    if "nc" in _CACHE:
        return _CACHE["nc"]
    nc = bacc.Bacc("TRN2", target_bir_lowering=False, debug=False, num_devices=NCORE)
    dr = {
        "xT": nc.dram_tensor("xT", [128, FT * S], BF16, kind="ExternalInput"),
        "wqk": nc.dram_tensor("wqk", [128, 3 * FT * 128], BF16, kind="ExternalInput"),
        "wv": nc.dram_tensor("wv", [128, FT * 192], BF16, kind="ExternalInput"),
        "woAB": nc.dram_tensor("woAB", [128, D], BF16, kind="ExternalInput"),
        "woC": nc.dram_tensor("woC", [65, D], BF16, kind="ExternalInput"),
        "bqk": nc.dram_tensor("bqk", [128, 3], F32, kind="ExternalInput"),
        "bv": nc.dram_tensor("bv", [128, 192], F32, kind="ExternalInput"),
        "mask": nc.dram_tensor("mask", [128, 128], BF16, kind="ExternalInput"),
    }
    y_dr = nc.dram_tensor("y", [S, D], F32, kind="ExternalOutput")
    with tile.TileContext(nc) as tc:
        _emit(nc, tc, dr, y_dr)
    nc.compile()
    _CACHE["nc"] = nc
    return nc


def prep_inputs(x, Wq, bq, Wk, bk, Wv, bv, Wo, bo):
    """Shard + pre-layout the full fp32 inputs into 8 per-core input maps."""
    in_maps = []
    mask = (np.arange(128)[:, None] <= np.arange(128)[None, :]).astype(NPBF)
    for c in range(NCORE):
        b, g = c // 4, c % 4
        hs = [3 * g, 3 * g + 1, 3 * g + 2]

        xT = np.ascontiguousarray(
            x[b].T.reshape(FT, 128, QC, 512).transpose(1, 2, 0, 3)
        )  # [128, QC, FT, 512] chunk-major

        def rows(W, h):
            return W[h * 64 : (h + 1) * 64]  # [64, D]

        G0 = np.concatenate([rows(Wq, hs[0]), rows(Wq, hs[1])], 0)  # [128, D]
        G1 = np.concatenate([rows(Wk, hs[0]), rows(Wk, hs[1])], 0)
        G2 = np.concatenate([rows(Wq, hs[2]), rows(Wk, hs[2])], 0)
        # wqk[p, g, f, m] = G_g[m, f*128+p]  (g-major so per-g DMAs are
        # contiguous per partition row)
        wqk = np.stack([G0, G1, G2], 0)  # [3, 128m, D]
        wqk = wqk.reshape(3, 128, FT, 128).transpose(3, 0, 2, 1)  # [128, 3, FT, 128]

        Vg = Wv[g * 192 : (g + 1) * 192]  # [192, D]
        wv_ = Vg.T.reshape(FT, 128, 192).transpose(1, 0, 2)  # [128, FT, 192]

        # out-proj rhs: rows = local head dims, cols = output features
        woAB = np.concatenate(
            [
                Wo[:, (3 * g + 0) * 64 : (3 * g + 1) * 64].T,
                Wo[:, (3 * g + 1) * 64 : (3 * g + 2) * 64].T,
            ],
            0,
        )  # [128, D]
        woC = np.zeros((65, D), np.float32)
        woC[0:64] = Wo[:, (3 * g + 2) * 64 : (3 * g + 3) * 64].T
        if g == 0:
            woC[64] = bo

        bqk_ = np.stack(
            [
                np.concatenate([bq[hs[0] * 64 : hs[0] * 64 + 64], bq[hs[1] * 64 : hs[1] * 64 + 64]]),
                np.concatenate([bk[hs[0] * 64 : hs[0] * 64 + 64], bk[hs[1] * 64 : hs[1] * 64 + 64]]),
                np.concatenate([bq[hs[2] * 64 : hs[2] * 64 + 64], bk[hs[2] * 64 : hs[2] * 64 + 64]]),
            ],
            1,
        ).astype(np.float32)  # [128, 3]

        bv_ = np.tile(bv[g * 192 : (g + 1) * 192][None, :], (128, 1)).astype(np.float32)

        in_maps.append(
            {
                "xT": xT.reshape(128, FT * S).astype(NPBF),
                "wqk": wqk.reshape(128, 3 * FT * 128).astype(NPBF),
                "wv": wv_.reshape(128, FT * 192).astype(NPBF),
                "woAB": woAB.astype(NPBF),
                "woC": woC.astype(NPBF),
                "bqk": bqk_,
                "bv": bv_,
                "mask": mask,
            }
        )
    return in_maps


def run_spmd(in_maps, trace=False, **kw):
    nc = _build()
    return run_bass_kernel_spmd(nc, in_maps, core_ids=list(range(NCORE)), trace=trace, **kw)


def gather(results):
    y = np.zeros((B, S, D), np.float32)
    for c in range(NCORE):
        y[c // 4] += results[c]["y"]
    return y


def kernel(x, Wq, bq, Wk, bk, Wv, bv, Wo, bo):
    args = [np.asarray(a, np.float32) for a in (x, Wq, bq, Wk, bk, Wv, bv, Wo, bo)]
    in_maps = prep_inputs(*args)
    last_err = None
    for _attempt in range(3):
        try:
            res = run_spmd(in_maps)
            return gather(res.results)
        except Exception as e:  # transient NRT/axon hiccups: retry
            last_err = e
            import time

            time.sleep(2.0)
    raise last_err
